# revision 2
# baseline (speedup 1.0000x reference)
"""Trainium2 Bass kernel for CAttention:
    k      = einsum('bcit,i->bct', x, alpha)
    scores = einsum('bct,ts,bds->bcd', k, Wc, k)
    att    = softmax(scores, axis=-1)
    out    = einsum('bci,bint->bcnt', att, x)

Sharding: data-parallel over batch B=64 across 8 NeuronCores (8 batches/core).

fp16 I/O version: x is cast to fp16 on the host (input DMA halves), the
channel-mix output is written as fp16 (output DMA halves), while the whole
score/softmax path accumulates in fp32.  Host-side numpy emulation of this
exact pipeline measures rel-err ~6.4e-3 against the fp32 reference.

Per-core layout (per batch b):
    X SBUF tile [128, 8192] fp16: partition p = j*8 + d  (j in [0,16) =
    n-chunk, d in [0,8) = channel), free q = n2*64 + t with n = j*128 + n2.

    k-path : PE-only (no DVE reduction).  alpha is folded into 16
             accumulating matmuls: chunk g covers n2 in [8g, 8g+8);
             stationary AC_g[(j,d), (n2l,d')] = delta_{dd'} alpha[j,g,n2l],
             moving X[:, 512g:512g+512] -> PSUM Y[(n2l,d'), (n2f,t)] fp32.
             Diagonal blocks (n2l==n2f) hold k partials by n2 mod 8; 8 tiny
             selector matmuls then fold them into k_ps[d, t].
    scores : kT via PE transpose; V = Wc @ kT; scores = kT.T @ V (fp32).
    softmax: stable (DVE -max as exp bias), unnormalized e in (0,1] fits
             fp16; 1/sum replicated via PE; normalization folded into the
             PSUM-evacuation scale.
    mix    : block-diag(e^T) [128,128] fp16 stationary, X fp16 moving,
             16 matmuls of 512 cols; evacuation alternates ACT/DVE in
             1024-wide slices with the per-partition 1/sum scale.

Emission interleaves the k-path of batch b+1 between the softmax chain and
the mix of batch b so the PE never waits on the (serial) softmax chain.
Input stream rides the SP HWDGE ring, output + constants the ACT ring.
"""

import sys

for _p in ("/opt/trn_rl_repo",):
    if _p not in sys.path:
        sys.path.insert(0, _p)

import numpy as np

B, C, N, T = 64, 8, 2048, 64
NCORES = 8
BS = B // NCORES          # batches per core
J = 16                    # n-chunks on partitions
N2 = N // J               # 128, n-extent in free dim
P = J * C                 # 128 partitions
F = N2 * T                # 8192 free elems
G = 16                    # k-path chunks (8 n2-values each)
NL = N2 // G              # 8, n2-local per chunk
QW = 512                  # matmul free width (one PSUM bank)
EV = 1024                 # evacuation slice width (two PSUM banks)

_PROGRAM_CACHE = {}


def _build_program():
    from contextlib import ExitStack

    import concourse.bacc as bacc
    from concourse import mybir, tile

    fp32 = mybir.dt.float32
    fp16 = mybir.dt.float16
    nc = bacc.Bacc("TRN2", target_bir_lowering=False, debug=False)

    xs = nc.dram_tensor("xs", [BS, C, N, T], fp16, kind="ExternalInput").ap()
    acw = nc.dram_tensor("acw", [P, G * 64], fp16, kind="ExternalInput").ap()
    # aux16 packed: rep16[0:128] (rows 0-7) | mask[128:256]
    aux16 = nc.dram_tensor("aux16", [P, 256], fp16, kind="ExternalInput").ap()
    # aux32 packed: wcT[0:64] (rows 0-63) | id64[64:128] (rows 0-63) |
    #               id8[128:136] (rows 0-7) | rep32[136:264] (rows 0-7)
    aux32 = nc.dram_tensor("aux32", [P, 264], fp32, kind="ExternalInput").ap()
    out = nc.dram_tensor("out", [BS, C, N, T], fp16, kind="ExternalOutput").ap()

    Exp = mybir.ActivationFunctionType.Exp
    Copy = mybir.ActivationFunctionType.Copy
    AX = mybir.AxisListType.X
    MAX = mybir.AluOpType.max
    MULT = mybir.AluOpType.mult

    with tile.TileContext(nc) as tc, ExitStack() as ctx:
        cpool = ctx.enter_context(tc.tile_pool(name="const", bufs=1))
        xpool = ctx.enter_context(tc.tile_pool(name="x", bufs=5))
        opool = ctx.enter_context(tc.tile_pool(name="o", bufs=4))
        spool = ctx.enter_context(tc.tile_pool(name="small", bufs=2))
        bdpool = ctx.enter_context(tc.tile_pool(name="bd", bufs=2))
        mixp = ctx.enter_context(tc.tile_pool(name="mixp", bufs=3, space="PSUM"))
        psmall = ctx.enter_context(tc.tile_pool(name="psmall", bufs=2, space="PSUM"))

        # constants ride the ACT HWDGE ring so batch 0's X read (SP ring)
        # starts concurrently
        ac_t = cpool.tile([P, G * 64], fp16)
        nc.scalar.dma_start(ac_t[:], acw)
        a16_t = cpool.tile([P, 256], fp16)
        nc.scalar.dma_start(a16_t[:], aux16)
        a32_t = cpool.tile([P, 264], fp32)
        nc.scalar.dma_start(a32_t[:], aux32)
        rep16_t = a16_t[:C, 0:128]
        mask_t = a16_t[:, 128:256]
        wcT_t = a32_t[:T, 0:64]
        id64_t = a32_t[:T, 64:128]
        id8_t = a32_t[:C, 128:136]
        rep32_t = a32_t[:C, 136:264]

        def phase_in(b):
            X = xpool.tile([P, F], fp16, tag="X")
            nc.sync.dma_start(
                X[:],
                xs[b].rearrange("d (j n2) t -> j d (n2 t)", j=J),
            )
            return X

        def phase_k(b, X):
            """PE k-path: 16 accumulating matmuls + 8 diagonal-fold matmuls."""
            y_ps = psmall.tile([T, QW], fp32, tag="ps")
            for g in range(G):
                nc.tensor.matmul(
                    y_ps[:],
                    lhsT=ac_t[:, g * 64 : (g + 1) * 64],
                    rhs=X[:, g * QW : (g + 1) * QW],
                    start=(g == 0),
                    stop=(g == G - 1),
                )
            y_sb = spool.tile([T, QW], fp32, tag="ysb")
            nc.scalar.copy(y_sb[:], y_ps[:])
            k_ps = psmall.tile([C, T], fp32, tag="ps")
            for l in range(NL):
                nc.tensor.matmul(
                    k_ps[:],
                    lhsT=id64_t[:, l * C : (l + 1) * C],
                    rhs=y_sb[:, l * T : (l + 1) * T],
                    start=(l == 0),
                    stop=(l == NL - 1),
                )
            # evacuate early (before the previous batch's mix evacuations are
            # queued on ACT) so the next chain's transpose isn't held up
            k_sb = spool.tile([C, T], fp32, tag="ksb")
            nc.scalar.copy(k_sb[:], k_ps[:])
            return k_sb

        def phase_chain(b, k_sb):
            """Tiny scores/softmax chain -> bd (fp16 block-diag) + rs scale."""
            kT_ps = psmall.tile([T, C], fp32, tag="ps")
            nc.tensor.transpose(kT_ps[:], k_sb[:], id8_t)
            kT_sb = spool.tile([T, C], fp32, tag="kTsb")
            nc.scalar.copy(kT_sb[:], kT_ps[:])

            v_ps = psmall.tile([T, C], fp32, tag="ps")
            nc.tensor.matmul(v_ps[:], lhsT=wcT_t, rhs=kT_sb[:], start=True, stop=True)
            v_sb = spool.tile([T, C], fp32, tag="vsb")
            nc.scalar.copy(v_sb[:], v_ps[:])

            sc_ps = psmall.tile([C, C], fp32, tag="ps")
            nc.tensor.matmul(sc_ps[:], lhsT=kT_sb[:], rhs=v_sb[:], start=True, stop=True)

            negmax = spool.tile([C, 1], fp32, tag="negmax")
            nc.vector.tensor_reduce(negmax[:], sc_ps[:], axis=AX, op=MAX, negate=True)
            e_sb = spool.tile([C, C], fp32, tag="esb")
            ssum = spool.tile([C, 1], fp32, tag="ssum")
            nc.scalar.activation(e_sb[:], sc_ps[:], Exp, bias=negmax[:], accum_out=ssum[:])
            rcp = spool.tile([C, 1], fp32, tag="rcp")
            nc.vector.reciprocal(rcp[:], ssum[:])

            rs_ps = psmall.tile([P, 1], fp32, tag="ps")
            nc.tensor.matmul(rs_ps[:], lhsT=rep32_t, rhs=rcp[:], start=True, stop=True)
            rs_sb = spool.tile([P, 1], fp32, tag="rssb")
            nc.scalar.copy(rs_sb[:], rs_ps[:])

            eT_ps = psmall.tile([C, C], fp32, tag="ps")
            nc.tensor.transpose(eT_ps[:], e_sb[:], id8_t)
            eT_sb = spool.tile([C, C], fp16, tag="eTsb")
            nc.scalar.copy(eT_sb[:], eT_ps[:])
            er_ps = psmall.tile([P, C], fp32, tag="ps")
            nc.tensor.matmul(er_ps[:], lhsT=rep16_t, rhs=eT_sb[:], start=True, stop=True)
            er_sb = spool.tile([P, C], fp16, tag="ersb")
            nc.scalar.copy(er_sb[:], er_ps[:])

            bd = bdpool.tile([P, P], fp16, tag="bd")
            nc.vector.tensor_tensor(
                out=bd[:].rearrange("p (j c) -> p j c", j=J),
                in0=mask_t.rearrange("p (j c) -> p j c", j=J),
                in1=er_sb[:].rearrange("p (x c) -> p x c", x=1).to_broadcast([P, J, C]),
                op=MULT,
            )
            return bd, rs_sb

        def phase_mix(b, X, bd, rs_sb):
            """Channel mix (fp16 PE) + normalized evacuation + DMA out."""
            FH = F // 2
            out_b = out[b].rearrange("c (j h n2) t -> h j c (n2 t)", j=J, h=2)
            for h in range(2):
                ost = opool.tile([P, FH], fp16, tag="ost")
                for s in range(FH // EV):
                    mp = mixp.tile([P, EV], fp32, tag="mix")
                    base = h * FH + s * EV
                    for q in range(EV // QW):
                        nc.tensor.matmul(
                            mp[:, q * QW : (q + 1) * QW],
                            lhsT=bd[:],
                            rhs=X[:, base + q * QW : base + (q + 1) * QW],
                            start=True,
                            stop=True,
                        )
                    dst = ost[:, s * EV : (s + 1) * EV]
                    if s % 2 == 0:
                        nc.scalar.activation(dst, mp[:], Copy, scale=rs_sb[:])
                    else:
                        nc.vector.tensor_scalar_mul(dst, mp[:], rs_sb[:])
                nc.scalar.dma_start(out_b[h], ost[:])

        # software-pipelined emission: k(b+1) sits between chain(b) and
        # mix(b) in the PE queue, so the PE streams X(b+1) while the softmax
        # chain of batch b ping-pongs through ACT/DVE
        X = [None] * BS
        X[0] = phase_in(0)
        k_sb = phase_k(0, X[0])
        for b in range(BS):
            bd, rs_sb = phase_chain(b, k_sb)
            if b + 1 < BS:
                X[b + 1] = phase_in(b + 1)
                k_sb = phase_k(b + 1, X[b + 1])
            phase_mix(b, X[b], bd, rs_sb)
            X[b] = None

    nc.compile()
    return nc


def _host_constants(Wc: np.ndarray, alpha: np.ndarray):
    # AC[(j*8+d), g*64 + n2l*8 + d'] = delta_{dd'} * alpha[j*128 + g*8 + n2l]
    a3 = np.asarray(alpha, dtype=np.float32).reshape(J, G, NL)
    ac = np.zeros((J, C, G, NL, C), dtype=np.float16)
    for d in range(C):
        ac[:, d, :, :, d] = a3
    ac = ac.reshape(P, G * 64)

    rep16 = np.tile(np.eye(C, dtype=np.float16), (1, J))          # [8, 128]
    mask = np.kron(
        np.eye(J, dtype=np.float16), np.ones((C, C), dtype=np.float16)
    )                                                              # [128, 128]
    aux16 = np.zeros((P, 256), dtype=np.float16)
    aux16[:C, 0:128] = rep16
    aux16[:, 128:256] = mask

    aux32 = np.zeros((P, 264), dtype=np.float32)
    aux32[:T, 0:64] = np.asarray(Wc, dtype=np.float32).T
    aux32[:T, 64:128] = np.eye(T, dtype=np.float32)
    aux32[:C, 128:136] = np.eye(C, dtype=np.float32)
    aux32[:C, 136:264] = np.tile(np.eye(C, dtype=np.float32), (1, J))
    return {
        "acw": np.ascontiguousarray(ac),
        "aux16": aux16,
        "aux32": aux32,
    }


def get_program():
    if "nc" not in _PROGRAM_CACHE:
        _PROGRAM_CACHE["nc"] = _build_program()
    return _PROGRAM_CACHE["nc"]


def run(x, Wc, alpha, trace=False, trace_kwargs=None):
    """Run on 8 cores; returns (full_output, BassKernelResults)."""
    from concourse.bass_utils import run_bass_kernel_spmd

    nc = get_program()
    consts = _host_constants(np.asarray(Wc), np.asarray(alpha))
    x16 = np.asarray(x, dtype=np.float16)
    in_maps = []
    for r in range(NCORES):
        m = {"xs": np.ascontiguousarray(x16[r * BS : (r + 1) * BS])}
        m.update(consts)
        in_maps.append(m)
    kw = {}
    if trace:
        kw["trace"] = True
        if trace_kwargs:
            kw.update(trace_kwargs)
    res = run_bass_kernel_spmd(nc, in_maps, list(range(NCORES)), **kw)
    out = np.concatenate([res.results[r]["out"] for r in range(NCORES)], axis=0)
    return out, res


def kernel(x, Wc, alpha):
    out, _ = run(x, Wc, alpha)
    return out.astype(np.float32)


# revision 3
# speedup vs baseline: 1.2069x; 1.2069x over previous
"""Trainium2 Bass kernel for CAttention:
    k      = einsum('bcit,i->bct', x, alpha)
    scores = einsum('bct,ts,bds->bcd', k, Wc, k)
    att    = softmax(scores, axis=-1)
    out    = einsum('bci,bint->bcnt', att, x)

Sharding: data-parallel over batch B=64 across 8 NeuronCores (8 batches/core).

fp16 I/O version: x is cast to fp16 on the host (input DMA halves), the
channel-mix output is written as fp16 (output DMA halves), while the whole
score/softmax path accumulates in fp32.  Host-side numpy emulation of this
exact pipeline measures rel-err ~8.5e-3 against the fp32 reference.

Per-core layout (per batch b):
    X SBUF tile [128, 8192] fp16: partition p = j*8 + d  (j in [0,16) =
    n-chunk, d in [0,8) = channel), free q = n2*64 + t with n = j*128 + n2.

    k-path : PE-only.  alpha is folded into 16 accumulating matmuls: chunk
             g covers n2 in [8g, 8g+8); stationary AC_g[(j,d), (n2l,d')] =
             delta_{dd'} alpha[j,g,n2l], moving X[:, 512g:512g+512].
             Even/odd chunks run CONCURRENTLY in PE column groups 0/1
             (psum base partition 0/64 -> col_grp), halving k-path wall
             time.  Diagonal blocks (n2l==n2f) hold k partials by n2 mod 8;
             8 selector matmuls (both groups summed at once) fold them
             into k_ps[d, t].
    scores : kT via PE transpose; V = Wc @ kT; scores = kT.T @ V (fp32).
    softmax: stable (DVE -max as exp bias), unnormalized e in (0,1] fits
             fp16; 1/sum replicated via PE; normalization folded into the
             PSUM-evacuation scale.
    mix    : block-diag(e^T) [128,128] fp16 stationary, X fp16 moving,
             16 matmuls of 512 cols; evacuation alternates ACT/DVE in
             1024-wide slices with the per-partition 1/sum scale.

DMA queue usage (per-queue HWDGE throughput caps at ~250 GB/s, HBM at
~358 GB/s per core, so three concurrent queues):
    sync  (HWDGE q1) : constants first, then input chunks 0-9  (10KB/part)
    gpsimd(SWDGE)    : input chunks 10-15 (6KB/part) + output slices 5-7
    scalar(HWDGE q10): output slices 0-4 (10KB/part)

Emission interleaves the k-path of batch b+1 between the softmax chain and
the mix of batch b so the PE never waits on the (serial) softmax chain.
"""

import sys

for _p in ("/opt/trn_rl_repo",):
    if _p not in sys.path:
        sys.path.insert(0, _p)

import numpy as np

B, C, N, T = 64, 8, 2048, 64
NCORES = 8
BS = B // NCORES          # batches per core
J = 16                    # n-chunks on partitions
N2 = N // J               # 128, n-extent in free dim
P = J * C                 # 128 partitions
F = N2 * T                # 8192 free elems
G = 16                    # k-path chunks (8 n2-values each)
NL = N2 // G              # 8, n2-local per chunk
QW = 512                  # matmul free width (one PSUM bank)
EV = 1024                 # evacuation slice width (two PSUM banks)
GH = 10                   # input-chunk split: sync ring gets chunks 0-9
FH = GH * QW              # 5120 cols on the sync ring
SH = 5                    # output-slice split: ACT ring gets slices 0-4
OH = SH * EV              # 5120 cols on the ACT ring

_PROGRAM_CACHE = {}


def _build_program():
    from contextlib import ExitStack

    import concourse.bacc as bacc
    from concourse import mybir, tile

    fp32 = mybir.dt.float32
    fp16 = mybir.dt.float16
    nc = bacc.Bacc("TRN2", target_bir_lowering=False, debug=False)

    xs = nc.dram_tensor("xs", [BS, C, N, T], fp16, kind="ExternalInput").ap()
    # aux16 packed: AC[0:1024] | idsel2[1024:1088] | rep16[1088:1216] (rows
    # 0-7) | mask[1216:1344]
    aux16 = nc.dram_tensor("aux16", [P, 1344], fp16, kind="ExternalInput").ap()
    # aux32 packed: wcT[0:64] (rows 0-63) | id8[64:72] (rows 0-7) |
    #               rep32[72:200] (rows 0-7)
    aux32 = nc.dram_tensor("aux32", [P, 200], fp32, kind="ExternalInput").ap()
    out = nc.dram_tensor("out", [BS, C, N, T], fp16, kind="ExternalOutput").ap()

    Exp = mybir.ActivationFunctionType.Exp
    Copy = mybir.ActivationFunctionType.Copy
    AX = mybir.AxisListType.X
    MAX = mybir.AluOpType.max
    MULT = mybir.AluOpType.mult

    with tile.TileContext(nc) as tc, ExitStack() as ctx:
        cpool = ctx.enter_context(tc.tile_pool(name="const", bufs=1))
        xpool = ctx.enter_context(tc.tile_pool(name="x", bufs=5))
        opool = ctx.enter_context(tc.tile_pool(name="o", bufs=3))
        spool = ctx.enter_context(tc.tile_pool(name="small", bufs=2))
        bdpool = ctx.enter_context(tc.tile_pool(name="bd", bufs=2))
        mixp = ctx.enter_context(tc.tile_pool(name="mixp", bufs=3, space="PSUM"))
        psmall = ctx.enter_context(tc.tile_pool(name="psmall", bufs=2, space="PSUM"))

        # constants ride the sync ring BEFORE X(0): wide lines, ~1.5us, so
        # the first chain isn't stalled on a slow trickle
        a16_t = cpool.tile([P, 1344], fp16)
        nc.sync.dma_start(a16_t[:], aux16)
        a32_t = cpool.tile([P, 200], fp32)
        nc.sync.dma_start(a32_t[:], aux32)
        ac_t = a16_t[:, 0:1024]
        idsel_t = a16_t[:, 1024:1088]
        rep16_t = a16_t[:C, 1088:1216]
        mask_t = a16_t[:, 1216:1344]
        wcT_t = a32_t[:T, 0:64]
        id8_t = a32_t[:C, 64:72]
        rep32_t = a32_t[:C, 72:200]

        def phase_in(b):
            X = xpool.tile([P, F], fp16, tag="X")
            src = xs[b].rearrange("d (j n2) t -> j d (n2 t)", j=J)
            nc.sync.dma_start(X[:, :FH], src[:, :, :FH])
            nc.gpsimd.dma_start(X[:, FH:], src[:, :, FH:])
            return X

        def phase_k(b, X):
            """PE k-path: 2x8 col-group-packed accumulating matmuls + fold."""
            yA = psmall.tile([P, QW], fp32, tag="ps")
            yB = psmall.tile([P, QW], fp32, tag="ps")
            for g in range(G):
                y = yA[0:T] if g % 2 == 0 else yB[T:P]
                nc.tensor.matmul(
                    y,
                    lhsT=ac_t[:, g * 64 : (g + 1) * 64],
                    rhs=X[:, g * QW : (g + 1) * QW],
                    start=(g < 2),
                    stop=(g >= G - 2),
                )
            y_sb = spool.tile([P, QW], fp16, tag="ysb")
            nc.scalar.copy(y_sb[0:T], yA[0:T])
            nc.vector.tensor_copy(y_sb[T:P], yB[T:P])
            k_ps = psmall.tile([C, T], fp32, tag="ps")
            for l in range(NL):
                nc.tensor.matmul(
                    k_ps[:],
                    lhsT=idsel_t[:, l * C : (l + 1) * C],
                    rhs=y_sb[:, l * T : (l + 1) * T],
                    start=(l == 0),
                    stop=(l == NL - 1),
                )
            # evacuate early (before the previous batch's mix evacuations are
            # queued) so the next chain's transpose isn't held up
            k_sb = spool.tile([C, T], fp32, tag="ksb")
            nc.vector.tensor_copy(k_sb[:], k_ps[:])
            return k_sb

        def phase_chain(b, k_sb):
            """Tiny scores/softmax chain -> bd (fp16 block-diag) + rs scale."""
            kT_ps = psmall.tile([T, C], fp32, tag="ps")
            nc.tensor.transpose(kT_ps[:], k_sb[:], id8_t)
            kT_sb = spool.tile([T, C], fp32, tag="kTsb")
            nc.vector.tensor_copy(kT_sb[:], kT_ps[:])

            v_ps = psmall.tile([T, C], fp32, tag="ps")
            nc.tensor.matmul(v_ps[:], lhsT=wcT_t, rhs=kT_sb[:], start=True, stop=True)
            v_sb = spool.tile([T, C], fp32, tag="vsb")
            nc.scalar.copy(v_sb[:], v_ps[:])

            sc_ps = psmall.tile([C, C], fp32, tag="ps")
            nc.tensor.matmul(sc_ps[:], lhsT=kT_sb[:], rhs=v_sb[:], start=True, stop=True)

            negmax = spool.tile([C, 1], fp32, tag="negmax")
            nc.vector.tensor_reduce(negmax[:], sc_ps[:], axis=AX, op=MAX, negate=True)
            e_sb = spool.tile([C, C], fp32, tag="esb")
            ssum = spool.tile([C, 1], fp32, tag="ssum")
            nc.scalar.activation(e_sb[:], sc_ps[:], Exp, bias=negmax[:], accum_out=ssum[:])
            rcp = spool.tile([C, 1], fp32, tag="rcp")
            nc.vector.reciprocal(rcp[:], ssum[:])

            rs_ps = psmall.tile([P, 1], fp32, tag="ps")
            nc.tensor.matmul(rs_ps[:], lhsT=rep32_t, rhs=rcp[:], start=True, stop=True)
            rs_sb = spool.tile([P, 1], fp32, tag="rssb")
            nc.scalar.copy(rs_sb[:], rs_ps[:])

            eT_ps = psmall.tile([C, C], fp32, tag="ps")
            nc.tensor.transpose(eT_ps[:], e_sb[:], id8_t)
            eT_sb = spool.tile([C, C], fp16, tag="eTsb")
            nc.scalar.copy(eT_sb[:], eT_ps[:])
            er_ps = psmall.tile([P, C], fp32, tag="ps")
            nc.tensor.matmul(er_ps[:], lhsT=rep16_t, rhs=eT_sb[:], start=True, stop=True)
            er_sb = spool.tile([P, C], fp16, tag="ersb")
            nc.scalar.copy(er_sb[:], er_ps[:])

            bd = bdpool.tile([P, P], fp16, tag="bd")
            nc.vector.tensor_tensor(
                out=bd[:].rearrange("p (j c) -> p j c", j=J),
                in0=mask_t.rearrange("p (j c) -> p j c", j=J),
                in1=er_sb[:].rearrange("p (x c) -> p x c", x=1).to_broadcast([P, J, C]),
                op=MULT,
            )
            return bd, rs_sb

        def phase_mix(b, X, bd, rs_sb):
            """Channel mix (fp16 PE) + normalized evacuation + DMA out."""
            out_b = out[b].rearrange("c (j n2) t -> j c (n2 t)", j=J)
            ost_a = opool.tile([P, OH], fp16, tag="osta")
            ost_b = opool.tile([P, F - OH], fp16, tag="ostb")
            for s in range(F // EV):
                mp = mixp.tile([P, EV], fp32, tag="mix")
                base = s * EV
                for q in range(EV // QW):
                    nc.tensor.matmul(
                        mp[:, q * QW : (q + 1) * QW],
                        lhsT=bd[:],
                        rhs=X[:, base + q * QW : base + (q + 1) * QW],
                        start=True,
                        stop=True,
                    )
                if s < SH:
                    dst = ost_a[:, base : base + EV]
                else:
                    dst = ost_b[:, base - OH : base - OH + EV]
                if s % 2 == 0:
                    nc.scalar.activation(dst, mp[:], Copy, scale=rs_sb[:])
                else:
                    nc.vector.tensor_scalar_mul(dst, mp[:], rs_sb[:])
                if s == SH - 1:
                    nc.scalar.dma_start(out_b[:, :, :OH], ost_a[:])
            nc.gpsimd.dma_start(out_b[:, :, OH:], ost_b[:])

        # software-pipelined emission: k(b+1) sits between chain(b) and
        # mix(b) in the PE queue, so the PE streams X(b+1) while the softmax
        # chain of batch b ping-pongs through ACT/DVE
        X = [None] * BS
        X[0] = phase_in(0)
        k_sb = phase_k(0, X[0])
        for b in range(BS):
            bd, rs_sb = phase_chain(b, k_sb)
            if b + 1 < BS:
                X[b + 1] = phase_in(b + 1)
                k_sb = phase_k(b + 1, X[b + 1])
            phase_mix(b, X[b], bd, rs_sb)
            X[b] = None

    nc.compile()
    return nc


def _host_constants(Wc: np.ndarray, alpha: np.ndarray):
    # AC[(j*8+d), g*64 + n2l*8 + d'] = delta_{dd'} * alpha[j*128 + g*8 + n2l]
    a3 = np.asarray(alpha, dtype=np.float32).reshape(J, G, NL)
    ac = np.zeros((J, C, G, NL, C), dtype=np.float16)
    for d in range(C):
        ac[:, d, :, :, d] = a3
    ac = ac.reshape(P, G * 64)

    # idsel2[p, l*8+d'] = 1 if p mod 64 == l*8+d'  (sums both col-groups)
    idsel2 = np.tile(np.eye(T, dtype=np.float16), (2, 1))          # [128, 64]
    rep16 = np.tile(np.eye(C, dtype=np.float16), (1, J))           # [8, 128]
    mask = np.kron(
        np.eye(J, dtype=np.float16), np.ones((C, C), dtype=np.float16)
    )                                                              # [128, 128]
    aux16 = np.zeros((P, 1344), dtype=np.float16)
    aux16[:, 0:1024] = ac
    aux16[:, 1024:1088] = idsel2
    aux16[:C, 1088:1216] = rep16
    aux16[:, 1216:1344] = mask

    aux32 = np.zeros((P, 200), dtype=np.float32)
    aux32[:T, 0:64] = np.asarray(Wc, dtype=np.float32).T
    aux32[:C, 64:72] = np.eye(C, dtype=np.float32)
    aux32[:C, 72:200] = np.tile(np.eye(C, dtype=np.float32), (1, J))
    return {
        "aux16": aux16,
        "aux32": aux32,
    }


def get_program():
    if "nc" not in _PROGRAM_CACHE:
        _PROGRAM_CACHE["nc"] = _build_program()
    return _PROGRAM_CACHE["nc"]


def run(x, Wc, alpha, trace=False, trace_kwargs=None):
    """Run on 8 cores; returns (full_output, BassKernelResults)."""
    from concourse.bass_utils import run_bass_kernel_spmd

    nc = get_program()
    consts = _host_constants(np.asarray(Wc), np.asarray(alpha))
    x16 = np.asarray(x, dtype=np.float16)
    in_maps = []
    for r in range(NCORES):
        m = {"xs": np.ascontiguousarray(x16[r * BS : (r + 1) * BS])}
        m.update(consts)
        in_maps.append(m)
    kw = {}
    if trace:
        kw["trace"] = True
        if trace_kwargs:
            kw.update(trace_kwargs)
    res = run_bass_kernel_spmd(nc, in_maps, list(range(NCORES)), **kw)
    out = np.concatenate([res.results[r]["out"] for r in range(NCORES)], axis=0)
    return out, res


def kernel(x, Wc, alpha):
    out, _ = run(x, Wc, alpha)
    return out.astype(np.float32)


# revision 5
# speedup vs baseline: 1.2340x; 1.0224x over previous
"""Trainium2 Bass kernel for CAttention:
    k      = einsum('bcit,i->bct', x, alpha)
    scores = einsum('bct,ts,bds->bcd', k, Wc, k)
    att    = softmax(scores, axis=-1)
    out    = einsum('bci,bint->bcnt', att, x)

Sharding: data-parallel over batch B=64 across 8 NeuronCores (8 batches/core).

fp16 I/O version: x is cast to fp16 on the host (input DMA halves), the
channel-mix output is written as fp16 (output DMA halves), while the whole
score/softmax path accumulates in fp32.  Host-side numpy emulation of this
exact pipeline measures rel-err ~8.5e-3 against the fp32 reference.

Per-core layout (per batch b):
    X SBUF tile [128, 8192] fp16: partition p = j*8 + d  (j in [0,16) =
    n-chunk, d in [0,8) = channel), free q = n2*64 + t with n = j*128 + n2.

    k-path : PE-only.  alpha is folded into 16 accumulating matmuls: chunk
             g covers n2 in [8g, 8g+8); stationary AC_g[(j,d), (n2l,d')] =
             delta_{dd'} alpha[j,g,n2l], moving X[:, 512g:512g+512].
             Even/odd chunks run CONCURRENTLY in PE column groups 0/1
             (psum base partition 0/64 -> col_grp), halving k-path wall
             time.  Diagonal blocks (n2l==n2f) hold k partials by n2 mod 8;
             8 selector matmuls (both groups summed at once) fold them
             into k_ps[d, t].
    scores : kT via PE transpose; V = Wc @ kT; scores = kT.T @ V (fp32).
    softmax: stable (DVE -max as exp bias); att = e * (1/sum) normalized
             on ACT before replication, so the mix stationary is真 att in
             fp16 and PSUM evacuation is a plain copy.
    mix    : block-diag(att^T) [128,128] fp16 stationary, X fp16 moving,
             16 matmuls of 512 cols; evacuation alternates ACT/DVE in
             1024-wide slices.

DMA queues (HWDGE rings get starved ~3x by SWDGE's fatter packets under
contention, and each queue alone caps well below the 358 GB/s HBM limit,
so all three are used):
    sync  (HWDGE) : constants, then input chunks 0-9 of batches 1..7
    gpsimd(SWDGE) : ALL of batch 0's X (critical path at startup), input
                    chunks 10-15 of batches 1..7, output slices 5-7
    scalar(HWDGE) : output slices 0-4

HAM note: the PE clock-gates to 1.2 GHz after ~3.4us of idling.  To keep
the matmul stream dense, the last 3 mix slices of batch b-1 are NOT
emitted with the mix head; their matmuls are interleaved one-by-one
between the softmax-chain PE ops of batch b, covering each cross-engine
wait with real work.
"""

import sys

for _p in ("/opt/trn_rl_repo",):
    if _p not in sys.path:
        sys.path.insert(0, _p)

import numpy as np

B, C, N, T = 64, 8, 2048, 64
NCORES = 8
BS = B // NCORES          # batches per core
J = 16                    # n-chunks on partitions
N2 = N // J               # 128, n-extent in free dim
P = J * C                 # 128 partitions
F = N2 * T                # 8192 free elems
G = 16                    # k-path chunks (8 n2-values each)
NL = N2 // G              # 8, n2-local per chunk
QW = 512                  # matmul free width (one PSUM bank)
EV = 1024                 # evacuation slice width (two PSUM banks)
GH = 10                   # input-chunk split: sync ring gets chunks 0-9
FH = GH * QW              # 5120 cols on the sync ring
SH = 5                    # output-slice split: ACT ring gets slices 0-4
OH = SH * EV              # 5120 cols on the ACT ring

_PROGRAM_CACHE = {}


def _build_program():
    from contextlib import ExitStack

    import concourse.bacc as bacc
    from concourse import mybir, tile

    fp32 = mybir.dt.float32
    fp16 = mybir.dt.float16
    nc = bacc.Bacc("TRN2", target_bir_lowering=False, debug=False)

    xs = nc.dram_tensor("xs", [BS, C, N, T], fp16, kind="ExternalInput").ap()
    # aux16 packed: AC[0:1024] | idsel2[1024:1088] | rep16[1088:1216] (rows
    # 0-7) | mask[1216:1344]
    aux16 = nc.dram_tensor("aux16", [P, 1344], fp16, kind="ExternalInput").ap()
    # aux32 packed: wcT[0:64] (rows 0-63) | id8[64:72] (rows 0-7)
    aux32 = nc.dram_tensor("aux32", [P, 72], fp32, kind="ExternalInput").ap()
    out = nc.dram_tensor("out", [BS, C, N, T], fp16, kind="ExternalOutput").ap()

    Exp = mybir.ActivationFunctionType.Exp
    Copy = mybir.ActivationFunctionType.Copy
    AX = mybir.AxisListType.X
    MAX = mybir.AluOpType.max
    MULT = mybir.AluOpType.mult

    with tile.TileContext(nc) as tc, ExitStack() as ctx:
        cpool = ctx.enter_context(tc.tile_pool(name="const", bufs=1))
        xpool = ctx.enter_context(tc.tile_pool(name="x", bufs=5))
        opool = ctx.enter_context(tc.tile_pool(name="o", bufs=3))
        spool = ctx.enter_context(tc.tile_pool(name="small", bufs=2))
        bdpool = ctx.enter_context(tc.tile_pool(name="bd", bufs=2))
        mixp = ctx.enter_context(tc.tile_pool(name="mixp", bufs=3, space="PSUM"))
        psmall = ctx.enter_context(tc.tile_pool(name="psmall", bufs=2, space="PSUM"))

        a16_t = cpool.tile([P, 1344], fp16)
        nc.sync.dma_start(a16_t[:], aux16)
        a32_t = cpool.tile([P, 72], fp32)
        nc.sync.dma_start(a32_t[:], aux32)
        ac_t = a16_t[:, 0:1024]
        idsel_t = a16_t[:, 1024:1088]
        rep16_t = a16_t[:C, 1088:1216]
        mask_t = a16_t[:, 1216:1344]
        wcT_t = a32_t[:T, 0:64]
        id8_t = a32_t[:C, 64:72]

        def phase_in(b):
            X = xpool.tile([P, F], fp16, tag="X")
            src = xs[b].rearrange("d (j n2) t -> j d (n2 t)", j=J)
            if b == 0:
                # batch 0 is on the critical path: the SWDGE queue wins the
                # per-packet round-robin, so it delivers X(0) fastest
                nc.gpsimd.dma_start(X[:], src)
            else:
                nc.sync.dma_start(X[:, :FH], src[:, :, :FH])
                nc.gpsimd.dma_start(X[:, FH:], src[:, :, FH:])
            return X

        def phase_k(b, X):
            """PE k-path: 2x8 col-group-packed accumulating matmuls + fold."""
            yA = psmall.tile([P, QW], fp32, tag="ps")
            yB = psmall.tile([P, QW], fp32, tag="ps")
            for g in range(G):
                y = yA[0:T] if g % 2 == 0 else yB[T:P]
                nc.tensor.matmul(
                    y,
                    lhsT=ac_t[:, g * 64 : (g + 1) * 64],
                    rhs=X[:, g * QW : (g + 1) * QW],
                    start=(g < 2),
                    stop=(g >= G - 2),
                )
            y_sb = spool.tile([P, QW], fp16, tag="ysb")
            nc.scalar.copy(y_sb[0:T], yA[0:T])
            nc.vector.tensor_copy(y_sb[T:P], yB[T:P])
            k_ps = psmall.tile([C, T], fp32, tag="ps")
            for l in range(NL):
                nc.tensor.matmul(
                    k_ps[:],
                    lhsT=idsel_t[:, l * C : (l + 1) * C],
                    rhs=y_sb[:, l * T : (l + 1) * T],
                    start=(l == 0),
                    stop=(l == NL - 1),
                )
            k_sb = spool.tile([C, T], fp32, tag="ksb")
            nc.vector.tensor_copy(k_sb[:], k_ps[:])
            return k_sb

        def phase_chain(b, k_sb, fillers):
            """Scores/softmax chain -> bd + plain-copy evacuation setup.

            `fillers` is a list of zero-arg closures, each emitting one PE
            matmul of the previous batch's deferred mix tail (plus its
            evacuation bookkeeping).  One is emitted before each chain PE
            op so the PE queue never sits idle on a cross-engine wait.
            """
            fill = iter(fillers)

            def f():
                nxt = next(fill, None)
                if nxt is not None:
                    nxt()

            f()
            kT_ps = psmall.tile([T, C], fp32, tag="ps")
            nc.tensor.transpose(kT_ps[:], k_sb[:], id8_t)
            kT_sb = spool.tile([T, C], fp32, tag="kTsb")
            nc.vector.tensor_copy(kT_sb[:], kT_ps[:])

            f()
            v_ps = psmall.tile([T, C], fp32, tag="ps")
            nc.tensor.matmul(v_ps[:], lhsT=wcT_t, rhs=kT_sb[:], start=True, stop=True)
            v_sb = spool.tile([T, C], fp32, tag="vsb")
            nc.scalar.copy(v_sb[:], v_ps[:])

            f()
            sc_ps = psmall.tile([C, C], fp32, tag="ps")
            nc.tensor.matmul(sc_ps[:], lhsT=kT_sb[:], rhs=v_sb[:], start=True, stop=True)

            negmax = spool.tile([C, 1], fp32, tag="negmax")
            nc.vector.tensor_reduce(negmax[:], sc_ps[:], axis=AX, op=MAX, negate=True)
            e_sb = spool.tile([C, C], fp32, tag="esb")
            ssum = spool.tile([C, 1], fp32, tag="ssum")
            nc.scalar.activation(e_sb[:], sc_ps[:], Exp, bias=negmax[:], accum_out=ssum[:])
            rcp = spool.tile([C, 1], fp32, tag="rcp")
            nc.vector.reciprocal(rcp[:], ssum[:])
            att_sb = spool.tile([C, C], fp32, tag="attsb")
            nc.scalar.activation(att_sb[:], e_sb[:], Copy, scale=rcp[:])

            f()
            aT_ps = psmall.tile([C, C], fp32, tag="ps")
            nc.tensor.transpose(aT_ps[:], att_sb[:], id8_t)
            aT_sb = spool.tile([C, C], fp16, tag="aTsb")
            nc.scalar.copy(aT_sb[:], aT_ps[:])

            f()
            ar_ps = psmall.tile([P, C], fp32, tag="ps")
            nc.tensor.matmul(ar_ps[:], lhsT=rep16_t, rhs=aT_sb[:], start=True, stop=True)
            ar_sb = spool.tile([P, C], fp16, tag="arsb")
            nc.scalar.copy(ar_sb[:], ar_ps[:])

            f()
            bd = bdpool.tile([P, P], fp16, tag="bd")
            nc.vector.tensor_tensor(
                out=bd[:].rearrange("p (j c) -> p j c", j=J),
                in0=mask_t.rearrange("p (j c) -> p j c", j=J),
                in1=ar_sb[:].rearrange("p (x c) -> p x c", x=1).to_broadcast([P, J, C]),
                op=MULT,
            )
            # drain any leftover fillers
            for nxt in fill:
                nxt()
            return bd

        def emit_mix_slice(X, bd, mp, dst, s, q):
            base = s * EV + q * QW
            nc.tensor.matmul(
                mp[:, q * QW : (q + 1) * QW],
                lhsT=bd[:],
                rhs=X[:, base : base + QW],
                start=True,
                stop=True,
            )
            if q == EV // QW - 1:
                if s % 2 == 0:
                    nc.scalar.copy(dst, mp[:])
                else:
                    nc.vector.tensor_copy(dst, mp[:])

        def phase_mix_head(b, X, bd):
            """Mix slices 0..SH-1 -> ost_a -> ACT-ring DMA.  Returns the
            deferred tail closures for slices SH..7 (emitted later)."""
            out_b = out[b].rearrange("c (j n2) t -> j c (n2 t)", j=J)
            ost_a = opool.tile([P, OH], fp16, tag="osta")
            ost_b = opool.tile([P, F - OH], fp16, tag="ostb")
            for s in range(SH):
                mp = mixp.tile([P, EV], fp32, tag="mix")
                for q in range(EV // QW):
                    emit_mix_slice(X, bd, mp, ost_a[:, s * EV : (s + 1) * EV], s, q)
            nc.scalar.dma_start(out_b[:, :, :OH], ost_a[:])

            fillers = []
            state = {}
            for s in range(SH, F // EV):
                for q in range(EV // QW):
                    def mk(s=s, q=q):
                        def emit():
                            if q == 0:
                                state["mp"] = mixp.tile(
                                    [P, EV], fp32, tag="mix", name="mp_tail"
                                )
                            dst = ost_b[:, s * EV - OH : (s + 1) * EV - OH]
                            emit_mix_slice(X, bd, state["mp"], dst, s, q)
                            if s == F // EV - 1 and q == EV // QW - 1:
                                nc.gpsimd.dma_start(out_b[:, :, OH:], ost_b[:])
                        return emit
                    fillers.append(mk())
            return fillers

        # software-pipelined emission:
        #   PE queue: k(0) | chain(0) | k(1) | head(0) | chain(1)+tail(0)
        #             | k(2) | head(1) | chain(2)+tail(1) | ... | tail(7)
        X = [None] * BS
        X[0] = phase_in(0)
        k_sb = phase_k(0, X[0])
        fillers = []
        for b in range(BS):
            bd = phase_chain(b, k_sb, fillers)
            if b + 1 < BS:
                X[b + 1] = phase_in(b + 1)
                k_sb = phase_k(b + 1, X[b + 1])
            fillers = phase_mix_head(b, X[b], bd)
            X[b] = None
        for nxt in fillers:
            nxt()

    nc.compile()
    return nc


def _host_constants(Wc: np.ndarray, alpha: np.ndarray):
    # AC[(j*8+d), g*64 + n2l*8 + d'] = delta_{dd'} * alpha[j*128 + g*8 + n2l]
    a3 = np.asarray(alpha, dtype=np.float32).reshape(J, G, NL)
    ac = np.zeros((J, C, G, NL, C), dtype=np.float16)
    for d in range(C):
        ac[:, d, :, :, d] = a3
    ac = ac.reshape(P, G * 64)

    # idsel2[p, l*8+d'] = 1 if p mod 64 == l*8+d'  (sums both col-groups)
    idsel2 = np.tile(np.eye(T, dtype=np.float16), (2, 1))          # [128, 64]
    rep16 = np.tile(np.eye(C, dtype=np.float16), (1, J))           # [8, 128]
    mask = np.kron(
        np.eye(J, dtype=np.float16), np.ones((C, C), dtype=np.float16)
    )                                                              # [128, 128]
    aux16 = np.zeros((P, 1344), dtype=np.float16)
    aux16[:, 0:1024] = ac
    aux16[:, 1024:1088] = idsel2
    aux16[:C, 1088:1216] = rep16
    aux16[:, 1216:1344] = mask

    aux32 = np.zeros((P, 72), dtype=np.float32)
    aux32[:T, 0:64] = np.asarray(Wc, dtype=np.float32).T
    aux32[:C, 64:72] = np.eye(C, dtype=np.float32)
    return {
        "aux16": aux16,
        "aux32": aux32,
    }


def get_program():
    if "nc" not in _PROGRAM_CACHE:
        _PROGRAM_CACHE["nc"] = _build_program()
    return _PROGRAM_CACHE["nc"]


def run(x, Wc, alpha, trace=False, trace_kwargs=None):
    """Run on 8 cores; returns (full_output, BassKernelResults)."""
    from concourse.bass_utils import run_bass_kernel_spmd

    nc = get_program()
    consts = _host_constants(np.asarray(Wc), np.asarray(alpha))
    x16 = np.asarray(x, dtype=np.float16)
    in_maps = []
    for r in range(NCORES):
        m = {"xs": np.ascontiguousarray(x16[r * BS : (r + 1) * BS])}
        m.update(consts)
        in_maps.append(m)
    kw = {}
    if trace:
        kw["trace"] = True
        if trace_kwargs:
            kw.update(trace_kwargs)
    res = run_bass_kernel_spmd(nc, in_maps, list(range(NCORES)), **kw)
    out = np.concatenate([res.results[r]["out"] for r in range(NCORES)], axis=0)
    return out, res


def kernel(x, Wc, alpha):
    out, _ = run(x, Wc, alpha)
    return out.astype(np.float32)
